# revision 50
# baseline (speedup 1.0000x reference)
"""2-layer GCN (message passing + BatchNorm + ReLU) on 8 Trainium2 NeuronCores.

Strategy (graph/data parallel, per sharding hint):
- Nodes sharded by dst across 8 cores (12500/core, padded to 12544 = 98 tiles
  of 128). Tiny weights replicated.
- Sym-norm folded into the data path: table rows are pre-scaled by dinv[src]
  (fused into the linear / BN-apply writes), aggregated sums are post-scaled
  by dinv[dst] (fused into the PSUM->SBUF copies via activation scale). The
  per-edge one-hot is then a PURE equality (slot id match), built BATCHED:
  one DVE tensor_tensor(is_equal) per (group, window) chunk produces all of
  the chunk's [128,128] one-hots at once. Self-loops use one shared identity
  matrix (no per-block DVE work at all).
- Aggregation: edges bucketed by (dst-tile, src-window); per 128-edge block,
  gather h[src] rows via dma_gather (256B rows), scatter via accumulating
  matmul in PSUM. int16 gather indices => 4 src windows of 25088.
- BN (training-mode batch stats): per-feature sum/sumsq via one accumulating
  z^T[z|1] matmul (diag = sumsq), cross-core AllReduce of [64,2], affine+ReLU
  applied per tile. Biases b1/b2 cancel under BN and are dropped.
- Layer 2 aggregates first (linearity: segsum(y@W2) = segsum(y)@W2), producing
  accum^T [64,128] per tile, then one matmul with W2 -> node-major output.
- Perf: x/W1 in bf16 (full-rate PE for the input linear); the per-layer table
  AllGather is split into two tile-range halves (windows 0-1 live in half A)
  so windows 0-1 gathers start while half B's collective is in flight; BN
  apply + ReLU + dinv staging run fully batched on the vector engine. The
  dominating cost is gpsimd dma_gather descriptor generation (~8ns/idx on the
  Q7; measured faster than ap_gather/gather_transpose/transpose-mode), which
  bounds this design at ~2.4ms of gpsimd busy time.
"""
import os
import numpy as np
from contextlib import ExitStack

N = 100000
F = 128          # input features
H = 64           # hidden
NCORES = 8
NS = 12500       # nodes per core
T = 98           # dst tiles per core (98*128 = 12544)
NSP = T * 128    # 12544
NP = NCORES * NSP  # padded table rows 100352
WIN = NP // 4    # 25088-row src windows (int16-safe, equal size)
W = 4            # src windows over NP
GRP = 7          # dst tiles per group (PSUM banks: 7 agg + 1 cov)
EPS = 1e-5

_CACHE = {}
TRACE = False     # test-harness hook: profiled run stashes results in LAST
LAST = None


def _build_bass(cap_ch, spans):
    """cap_ch: int array [NG, W] — 128-edge block counts per (tile-group,
    window) chunk, identical across cores. spans[gi, w, tg] = (b0, b1) union
    block range of tile tg's edges within the chunk, across cores."""
    import concourse.bass as bass
    import concourse.tile as tile
    from concourse import bacc, mybir

    f32 = mybir.dt.float32
    f16 = mybir.dt.float16
    bf16 = mybir.dt.bfloat16
    i16 = mybir.dt.int16

    NGB = int(cap_ch.sum())         # gather blocks (no self blocks)
    NI = NGB * 128                  # gathered edge slots
    TA = 49                         # tiles in table half A (half B: T - TA)

    nc = bacc.Bacc("TRN2", target_bir_lowering=False, debug=False,
                   num_devices=NCORES)

    NG = T // GRP
    xT_in = nc.dram_tensor("xT", [F, NSP], bf16, kind="ExternalInput").ap()
    idx_in = nc.dram_tensor("idx", [128, NI // 16], i16, kind="ExternalInput").ap()
    dstl_in = nc.dram_tensor("dstl", [128, NGB], f32, kind="ExternalInput").ap()
    dinv_in = nc.dram_tensor("dinv", [128, T], f32, kind="ExternalInput").ap()
    w1_in = nc.dram_tensor("w1", [F, H], bf16, kind="ExternalInput").ap()
    w2_in = nc.dram_tensor("w2", [H, H], f32, kind="ExternalInput").ap()
    bnp_in = nc.dram_tensor("bnp", [H, 4], f32, kind="ExternalInput").ap()
    out_ext = nc.dram_tensor("out", [NSP, H], f32, kind="ExternalOutput").ap()

    # table split in two tile-range halves so gathers on half A (windows 0-1)
    # can start while half B's AllGather is still in flight
    ag_a = nc.dram_tensor("ag_a", [128, TA * F], f16).ap()
    ag_b = nc.dram_tensor("ag_b", [128, (T - TA) * F], f16).ap()
    table_a = nc.dram_tensor("table_a", [NP // 2, F], f16, addr_space="Shared").ap()
    table_b = nc.dram_tensor("table_b", [NP // 2, F], f16, addr_space="Shared").ap()
    # layer-2 tables are double-buffered: they receive PRE-BN z*dinv rows
    # (dinv stashed in col H) all-gathered DURING layer-1 aggregation; BN+relu
    # is applied per gathered chunk instead (relu(g*sc + of*dinv), dinv >= 0)
    ag_a2 = nc.dram_tensor("ag_a2", [128, TA * F], f16).ap()
    ag_b2 = nc.dram_tensor("ag_b2", [128, (T - TA) * F], f16).ap()
    table_a2 = nc.dram_tensor("table_a2", [NP // 2, F], f16, addr_space="Shared").ap()
    table_b2 = nc.dram_tensor("table_b2", [NP // 2, F], f16, addr_space="Shared").ap()
    ar_in = nc.dram_tensor("ar_in", [H, 2], f32).ap()
    ar_out = nc.dram_tensor("ar_out", [H, 2], f32, addr_space="Shared").ap()

    groups = [list(range(g, min(g + GRP, T))) for g in range(0, T, GRP)]

    # canonical block order: for g: for w: cap_ch[gi][w] blocks (chunk-packed)
    idx_off = {}    # (gi, w) -> (block offset, nblocks)
    goff = 0
    for gi, g in enumerate(groups):
        for w in range(W):
            nb = int(cap_ch[gi][w])
            idx_off[(gi, w)] = (goff, nb)
            goff += nb
    assert goff == NGB

    with tile.TileContext(nc) as tc, ExitStack() as ctx:
        const = ctx.enter_context(tc.tile_pool(name="const", bufs=1))
        big = ctx.enter_context(tc.tile_pool(name="big", bufs=1))
        gp = ctx.enter_context(tc.tile_pool(name="gp", bufs=6))
        ohp = ctx.enter_context(tc.tile_pool(name="ohp", bufs=3))
        ztp = ctx.enter_context(tc.tile_pool(name="ztp", bufs=3))
        ytp = ctx.enter_context(tc.tile_pool(name="ytp", bufs=3))
        smp = ctx.enter_context(tc.tile_pool(name="smp", bufs=2))
        ps_agg = ctx.enter_context(tc.tile_pool(name="ps_agg", bufs=GRP, space="PSUM"))
        ps_cov = ctx.enter_context(tc.tile_pool(name="ps_cov", bufs=1, space="PSUM"))

        # ---- constants / loads
        # qualified-label iota: col c of slice tg holds tg*128 + c, matching
        # the dq = tg*128 + slot one-hot labels
        iota_q = const.tile([128, GRP * 128], f32)
        nc.gpsimd.iota(iota_q[:], pattern=[[1, GRP * 128]], base=0,
                       channel_multiplier=0,
                       allow_small_or_imprecise_dtypes=True)
        # identity [128,128] f16 for self blocks: (c - p == 0)
        iota_d = const.tile([128, 128], f32)
        nc.gpsimd.iota(iota_d[:], pattern=[[1, 128]], base=0, channel_multiplier=-1,
                       allow_small_or_imprecise_dtypes=True)
        ident = const.tile([128, 128], f16)
        nc.vector.tensor_scalar(ident[:], iota_d[:], 0.0, None,
                                op0=mybir.AluOpType.is_equal)
        # diagmask[f, c] = (c - f == 0), [64, 66]; cols 0..63 also serve as I64
        iota_cm = const.tile([H, 66], f32)
        nc.gpsimd.iota(iota_cm[:], pattern=[[1, 66]], base=0, channel_multiplier=-1,
                       allow_small_or_imprecise_dtypes=True)
        diagmask = const.tile([H, 66], f32)
        nc.vector.tensor_scalar(diagmask[:], iota_cm[:], 0.0, None,
                                op0=mybir.AluOpType.is_equal)
        ones1 = const.tile([1, 128], f32)
        nc.vector.memset(ones1[:], 1.0)

        w1_t = const.tile([F, H], bf16)
        nc.sync.dma_start(w1_t[:], w1_in)
        w2_t = const.tile([H, H], f32)
        nc.sync.dma_start(w2_t[:], w2_in)
        bnp_t = const.tile([H, 4], f32)
        nc.sync.dma_start(bnp_t[:], bnp_in)
        dinv_t = const.tile([128, T], f32)
        nc.sync.dma_start(dinv_t[:], dinv_in)
        xT_t = big.tile([F, NSP], bf16, tag="xT")
        nc.sync.dma_start(xT_t[:], xT_in)
        idx_t = big.tile([128, NI // 16], i16, tag="idx")
        nc.sync.dma_start(idx_t[:], idx_in)
        dstl_t = big.tile([128, NGB], f32, tag="dstl")
        nc.sync.dma_start(dstl_t[:], dstl_in)

        tstage = big.tile([128, T, F], f16, tag="tstage")   # table staging
        nc.vector.memset(tstage[:], 0.0)
        nc.vector.tensor_copy(tstage[:, :, H:H + 1], dinv_t[:, :, None])
        z_sb = big.tile([128, T, 66], f32, tag="zsb")       # agg out + ones col
        nc.vector.memset(z_sb[:, :, 64:66], 0.0)
        nc.vector.memset(z_sb[:, :, 64:65], 1.0)

        def ag_half(half, layer=1, part="both"):
            """DMA one tile-range half of tstage out, AllGather it into its
            half table. Half-table row index: r = c*6272 + p*49 + t_local.
            part="dma"/"cc" splits emission: the staging DMA must be emitted
            while tstage still holds the values to ship, but the collective
            trigger (a gpsimd instruction) can be deferred so it doesn't
            head-of-line block the gather stream while its input lands."""
            if layer == 1:
                ag, tbl = (ag_a, table_a) if half == 0 else (ag_b, table_b)
            else:
                ag, tbl = (ag_a2, table_a2) if half == 0 else (ag_b2, table_b2)
            t0, t1 = (0, TA) if half == 0 else (TA, T)
            if part in ("both", "dma"):
                nc.sync.dma_start(ag,
                                  tstage[:, t0:t1, :].rearrange("p t f -> p (t f)"))
            if part in ("both", "cc"):
                nc.gpsimd.collective_compute(
                    "AllGather", mybir.AluOpType.bypass,
                    replica_groups=[list(range(NCORES))],
                    ins=[ag], outs=[tbl],
                )

        def aggregate(layer):
            """Gather + equality-one-hot matmul aggregation.
            layer 1: psum_t [128,64] node-major (lhsT=onehot).
            layer 2: accumT [64,128] (lhsT=messages), then @W2.
            Self-loop: one block per tile, lhsT/rhs = shared identity, rhs/lhsT
            from tstage (rows already dinv[src]-scaled)."""
            cov = ps_cov.tile([H, 66], f32, tag="cov")
            first = [True] * T

            def mm(t, psum_t, layer, oh, rhs_msg, stop):
                if t not in psum_t:
                    psum_t[t] = ps_agg.tile(
                        [128, H] if layer == 1 else [H, 128], f32,
                        tag="agg", name=f"agg_l{layer}_t{t}")
                if layer == 1:
                    nc.tensor.matmul(psum_t[t][:], oh, rhs_msg,
                                     start=first[t], stop=stop)
                else:
                    nc.tensor.matmul(psum_t[t][:], rhs_msg, oh,
                                     start=first[t], stop=stop)
                first[t] = False

            for gi, g in enumerate(groups):
                psum_t = {}
                for w in range(W):
                    o, nb = idx_off[(gi, w)]
                    if nb == 0:
                        continue
                    gt = gp.tile([128, nb, F], f16, tag="g",
                                 name=f"g_l{layer}_{gi}_{w}")
                    if layer == 1:
                        tbl = table_a if w < 2 else table_b
                    else:
                        tbl = table_a2 if w < 2 else table_b2
                    w2 = w % 2
                    assert nb <= 32                 # HW gather cap: 4096 idxs
                    # static full-count gather (runtime reg truncation measured
                    # slower: per-chunk reg_load WAR gating outweighed the
                    # ~4.6% descriptor saving)
                    nc.gpsimd.dma_gather(
                        gt[:, 0:nb, :], tbl[w2 * WIN:(w2 + 1) * WIN, :],
                        idx_t[:, o * 8:(o + nb) * 8],
                        num_idxs=nb * 128, num_idxs_reg=nb * 128,
                        elem_size=F, elem_step=F, single_packet=False)
                    if layer == 2:
                        # per-chunk BN+relu on the raw gathered rows:
                        # msg = relu(g*sc + of*dinv[src]) with dinv from col H
                        msg = ytp.tile([128, nb, H], f16, tag="msg",
                                       name=f"msg_{gi}_{w}")
                        tm2 = ytp.tile([128, nb, H], f16, tag="msg2",
                                       name=f"tm2_{gi}_{w}")
                        nc.vector.tensor_tensor(
                            out=msg[:], in0=gt[:, :, 0:H],
                            in1=bcs[:, None, 0:H].broadcast_to([128, nb, H]),
                            op=mybir.AluOpType.mult)
                        nc.vector.tensor_tensor(
                            out=tm2[:],
                            in0=bcs[:, None, H:128].broadcast_to([128, nb, H]),
                            in1=gt[:, :, H:H + 1].broadcast_to([128, nb, H]),
                            op=mybir.AluOpType.mult)
                        nc.vector.tensor_tensor(
                            out=msg[:], in0=msg[:], in1=tm2[:],
                            op=mybir.AluOpType.add)
                        nc.vector.tensor_scalar(msg[:], msg[:], 0.0, None,
                                                op0=mybir.AluOpType.max)
                    # per-tile one-hots over the tile's union block span;
                    # qualified labels keep other tiles'/pad edges at zero
                    for tg in range(len(g)):
                        b0 = int(spans[gi, w, tg, 0])
                        b1 = int(spans[gi, w, tg, 1])
                        if b1 <= b0:
                            continue
                        sp = b1 - b0
                        oh = ohp.tile([128, sp, 128], f16, tag="oh",
                                      name=f"oh_{gi}_{w}_{tg}")
                        nc.vector.tensor_tensor(
                            out=oh[:],
                            in0=iota_q[:, None, tg * 128:(tg + 1) * 128]
                                .broadcast_to([128, sp, 128]),
                            in1=dstl_t[:, o + b0:o + b1, None]
                                .broadcast_to([128, sp, 128]),
                            op=mybir.AluOpType.is_equal)
                        for b in range(b0, b1):
                            rhs = (gt[:, b, 0:H] if layer == 1
                                   else msg[:, b, :])
                            mm(g[tg], psum_t, layer, oh[:, b - b0, :],
                               rhs, stop=False)
                    if layer == 1 and gi == 7 and w == 0:
                        ag_half(0, layer=2, part="cc")
                    if layer == 2 and gi == 0 and w == 0:
                        ag_half(1, layer=2, part="cc")
                for t in g:
                    # self block: identity one-hot, own (scaled) table rows
                    mm(t, psum_t, layer, ident[:], tstage[:, t, 0:H],
                       stop=True)
                for t in g:
                    if layer == 1:
                        nc.scalar.activation(z_sb[:, t, 0:H], psum_t[t][:],
                                             mybir.ActivationFunctionType.Copy,
                                             scale=dinv_t[:, t:t + 1])
                    else:
                        zT = ztp.tile([H, 128], f32, tag="zT",
                                      name=f"zT_{t}")
                        nc.scalar.activation(zT[:], psum_t[t][:],
                                             mybir.ActivationFunctionType.Copy)
                        o2 = ps_agg.tile([128, H], f32, tag="agg",
                                         name=f"o2_{t}")
                        nc.tensor.matmul(o2[:], zT[:], w2_t[:],
                                         start=True, stop=True)
                        nc.scalar.activation(z_sb[:, t, 0:H], o2[:],
                                             mybir.ActivationFunctionType.Copy,
                                             scale=dinv_t[:, t:t + 1])
                    # stats: cov += z^T @ [z | 1]; diag -> sumsq, col 64 -> sum
                    nc.tensor.matmul(cov[:], z_sb[:, t, 0:H], z_sb[:, t, :],
                                     start=(t == 0), stop=(t == T - 1))
                if layer == 1:
                    g0 = g[0]
                    nc.vector.tensor_tensor(
                        out=tstage[:, g0:g0 + len(g), 0:H],
                        in0=z_sb[:, g0:g0 + len(g), 0:H],
                        in1=dinv_t[:, g0:g0 + len(g), None]
                            .broadcast_to([128, len(g), H]),
                        op=mybir.AluOpType.mult)
                    if gi == 6:
                        ag_half(0, layer=2, part="dma")
                    if gi == len(groups) - 1:
                        arbox.append(stats_ar(cov))
                        ag_half(1, layer=2, part="dma")
            return cov

        def stats_ar(cov):
            """Reduce cov to [sum | sumsq] and AllReduce across cores."""
            cov_sb = smp.tile([H, 66], f32, tag="covsb")
            nc.vector.tensor_copy(cov_sb[:], cov[:])
            stats = smp.tile([H, 2], f32, tag="stats")
            nc.vector.tensor_copy(stats[:, 0:1], cov_sb[:, 64:65])
            masked = smp.tile([H, 66], f32, tag="masked")
            nc.vector.tensor_tensor(out=masked[:], in0=cov_sb[:], in1=diagmask[:],
                                    op=mybir.AluOpType.mult)
            nc.vector.tensor_reduce(out=stats[:, 1:2], in_=masked[:],
                                    op=mybir.AluOpType.add,
                                    axis=mybir.AxisListType.X)
            nc.sync.dma_start(ar_in, stats[:])
            nc.gpsimd.collective_compute(
                "AllReduce", mybir.AluOpType.add,
                replica_groups=[list(range(NCORES))],
                ins=[ar_in], outs=[ar_out])
            ar_sb = smp.tile([H, 2], f32, tag="arsb")
            nc.sync.dma_start(ar_sb[:], ar_out)
            return ar_sb

        def bn_rest(ar_sb, pcol):
            """From all-reduced stats, build bcast psum [128,128]:
            cols 0:64 = scale, 64:128 = offset (per-feature)."""
            m_p = smp.tile([H, 1], f32, tag="m_p")
            nc.vector.tensor_scalar(m_p[:], ar_sb[:, 0:1], 1.0 / N, None,
                                    op0=mybir.AluOpType.mult)
            q_p = smp.tile([H, 1], f32, tag="q_p")
            nc.vector.tensor_scalar(q_p[:], ar_sb[:, 1:2], 1.0 / N, None,
                                    op0=mybir.AluOpType.mult)
            var_p = smp.tile([H, 1], f32, tag="var_p")
            nc.vector.tensor_tensor(out=var_p[:], in0=m_p[:], in1=m_p[:],
                                    op=mybir.AluOpType.mult)
            nc.vector.tensor_tensor(out=var_p[:], in0=q_p[:], in1=var_p[:],
                                    op=mybir.AluOpType.subtract)
            nc.vector.tensor_scalar(var_p[:], var_p[:], EPS, None,
                                    op0=mybir.AluOpType.add)
            std_p = smp.tile([H, 1], f32, tag="std_p")
            nc.scalar.activation(std_p[:], var_p[:],
                                 mybir.ActivationFunctionType.Sqrt)
            rstd_p = smp.tile([H, 1], f32, tag="rstd_p")
            nc.vector.reciprocal(rstd_p[:], std_p[:])
            so_p = smp.tile([H, 2], f32, tag="so_p")
            nc.vector.tensor_tensor(out=so_p[:, 0:1], in0=rstd_p[:],
                                    in1=bnp_t[:, pcol:pcol + 1],
                                    op=mybir.AluOpType.mult)
            nc.vector.tensor_tensor(out=so_p[:, 1:2], in0=m_p[:], in1=so_p[:, 0:1],
                                    op=mybir.AluOpType.mult)
            nc.vector.tensor_tensor(out=so_p[:, 1:2],
                                    in0=bnp_t[:, pcol + 1:pcol + 2],
                                    in1=so_p[:, 1:2],
                                    op=mybir.AluOpType.subtract)
            tr_s = ps_agg.tile([1, H], f32, tag="agg")
            nc.tensor.transpose(tr_s[:], so_p[:, 0:1], diagmask[:, 0:64])
            tr_o = ps_agg.tile([1, H], f32, tag="agg")
            nc.tensor.transpose(tr_o[:], so_p[:, 1:2], diagmask[:, 0:64])
            sc_row = smp.tile([1, H], f32, tag="sc_row")
            nc.vector.tensor_copy(sc_row[:], tr_s[:])
            of_row = smp.tile([1, H], f32, tag="of_row")
            nc.vector.tensor_copy(of_row[:], tr_o[:])
            bc = ps_cov.tile([128, 128], f32, tag="cov")
            nc.tensor.matmul(bc[:, 0:H], ones1[:], sc_row[:],
                             start=True, stop=True)
            nc.tensor.matmul(bc[:, H:128], ones1[:], of_row[:],
                             start=True, stop=True)
            return bc

        # ======== layer 1 ========
        for ha, (h0, h1) in enumerate(((0, TA), (TA, T))):
            for t0 in range(h0, h1, 4):
                k = min(4, h1 - t0)
                mm = ps_agg.tile([128, 4, H], f32, tag="agg", name=f"xmm_{t0}")
                for b in range(k):
                    t = t0 + b
                    nc.tensor.matmul(mm[:, b, :], xT_t[:, t * 128:(t + 1) * 128],
                                     w1_t[:], start=True, stop=True)
                # stage rows pre-scaled by dinv[node]
                nc.vector.tensor_tensor(
                    out=tstage[:, t0:t0 + k, 0:H], in0=mm[:, 0:k, :],
                    in1=dinv_t[:, t0:t0 + k, None].broadcast_to([128, k, H]),
                    op=mybir.AluOpType.mult)
            ag_half(ha)
        arbox = []
        aggregate(1)          # also stages raw z and fires AG2A/AR1/AG2B
        bc1 = bn_rest(arbox.pop(), 0)
        bcs = smp.tile([128, 128], f32, tag="bcs")
        nc.vector.tensor_copy(bcs[:], bc1[:])
        BNC = 7                                   # tiles per batched BN op
        # BN affine + relu + dinv[src] pre-scale, all batched on vector
        # (relu(y)*dinv == relu(y*dinv) since dinv >= 0); per half so the
        # half-A AllGather overlaps half-B's BN apply.
        for ha, (h0, h1) in enumerate(((0, TA), (TA, T))):
            for bi, t0 in enumerate(range(h0, h1, BNC)):
                k = min(BNC, h1 - t0)
                y = ytp.tile([128, BNC, H], f32, tag="y")
                sc_b = bc1[:, None, 0:H].broadcast_to([128, k, H])
                of_b = bc1[:, None, H:128].broadcast_to([128, k, H])
                nc.vector.tensor_tensor(out=y[:, 0:k, :],
                                        in0=z_sb[:, t0:t0 + k, 0:H],
                                        in1=sc_b, op=mybir.AluOpType.mult)
                nc.vector.tensor_tensor(out=y[:, 0:k, :], in0=y[:, 0:k, :],
                                        in1=of_b, op=mybir.AluOpType.add)
                if bi % 2 == 0:
                    # even batches: relu + dinv pre-scale on scalar so the
                    # vector engine streams ahead to the next batch
                    for b in range(k):
                        t = t0 + b
                        nc.scalar.activation(tstage[:, t, 0:H], y[:, b, :],
                                             mybir.ActivationFunctionType.Relu,
                                             scale=dinv_t[:, t:t + 1])
                else:
                    nc.vector.tensor_scalar(y[:, 0:k, :], y[:, 0:k, :], 0.0,
                                            None, op0=mybir.AluOpType.max)
                    nc.vector.tensor_tensor(
                        out=tstage[:, t0:t0 + k, 0:H], in0=y[:, 0:k, :],
                        in1=dinv_t[:, t0:t0 + k, None].broadcast_to([128, k, H]),
                        op=mybir.AluOpType.mult)
        # ======== layer 2 ========
        cov2 = aggregate(2)
        bc2 = bn_rest(stats_ar(cov2), 2)
        out_v = out_ext.rearrange("(t p) f -> t p f", p=128)
        for t0 in range(0, T, BNC):
            k = min(BNC, T - t0)
            y = ytp.tile([128, BNC, H], f32, tag="y")
            sc_b = bc2[:, None, 0:H].broadcast_to([128, k, H])
            of_b = bc2[:, None, H:128].broadcast_to([128, k, H])
            nc.vector.tensor_tensor(out=y[:, 0:k, :], in0=z_sb[:, t0:t0 + k, 0:H],
                                    in1=sc_b, op=mybir.AluOpType.mult)
            nc.vector.tensor_tensor(out=y[:, 0:k, :], in0=y[:, 0:k, :],
                                    in1=of_b, op=mybir.AluOpType.add)
            yo = ytp.tile([128, BNC, H], f32, tag="yo")
            nc.scalar.activation(yo[:, 0:k, :], y[:, 0:k, :],
                                 mybir.ActivationFunctionType.Relu)
            nc.sync.dma_start(out_v[t0:t0 + k].rearrange("t p f -> p t f"),
                              yo[:, 0:k, :])

    nc.compile()
    return nc


def _preprocess(x, edge_index):
    """Shard + bucket edges; build per-core device arrays.

    Nodes are permuted across (core, tile, slot) by snake-dealing in
    decreasing in-degree order, equalizing per-tile edge loads so the
    shared-across-cores bucket caps carry minimal padding. Self-loops are
    NOT bucketed: each tile gets one identity-matmul self block. Sym-norm
    is folded into per-node dinv scales (table rows pre-scaled by dinv[src],
    aggregated output post-scaled by dinv[dst])."""
    src = np.asarray(edge_index[0], dtype=np.int64)
    dst = np.asarray(edge_index[1], dtype=np.int64)
    deg = (np.bincount(dst, minlength=N) + 1).astype(np.float64)  # + self loop
    dinv = 1.0 / np.sqrt(deg)

    # ---- balance: node -> padded global position (tile*128 + slot)
    NT = NCORES * T                               # 784 tiles
    order = np.argsort(-deg, kind="stable")
    pos = np.empty(N, dtype=np.int64)
    for r in range((N + NT - 1) // NT):
        chunk = order[r * NT:(r + 1) * NT]
        tiles = np.arange(len(chunk))
        if r % 2 == 1:
            tiles = NT - 1 - tiles
        pos[chunk] = tiles * 128 + r
    # split-table row of a node (half A: t<49, half B: t>=49):
    # r_half = c*6272 + slot*49 + t_local; windows 0-1 in A, 2-3 in B
    TA = 49
    tile_of = pos >> 7
    slot_of = pos & 127
    core_of = tile_of // T
    t_of = tile_of - core_of * T
    in_b = t_of >= TA
    t_loc = np.where(in_b, t_of - TA, t_of)
    half_row = core_of * (TA * 128) + slot_of * TA + t_loc
    sp = half_row[src]                             # row within half table
    sw = (in_b[src].astype(np.int64) * 2) + sp // WIN  # window 0-3
    dp = pos[dst]
    core = dp // NSP
    local = dp - core * NSP
    t_arr = local >> 7
    dl_arr = (local & 127).astype(np.float32)
    w_arr = sw
    i_arr = (sp % WIN).astype(np.int16)

    # chunk-packed buckets: chunk = (group gi, window w); edges packed
    # tile-ordered with pads only at the chunk tail. Block quantization then
    # amortizes over ~2300-edge chunks (~9% pad) instead of ~320-edge
    # (tile, w) buckets (~20% pad). Boundary blocks span 2 tiles; the matmul
    # schedule uses the UNION over cores of each tile's block range, with
    # tile-qualified one-hot labels dq = tg*128 + slot.
    NG = T // GRP                                  # 14 groups
    g_arr = t_arr // GRP
    tg_arr = t_arr % GRP
    dq_arr = (tg_arr * 128 + (local & 127)).astype(np.float32)
    key = (((core * NG + g_arr) * W + w_arr) * GRP + tg_arr).astype(np.int64)
    cnt_tg = np.bincount(key, minlength=NCORES * NG * W * GRP) \
        .reshape(NCORES, NG, W, GRP)
    cnt_ch = cnt_tg.sum(axis=3)                    # [c, NG, W]
    cap_ch = ((cnt_ch.max(axis=0) + 127) // 128).astype(np.int64)  # [NG, W]
    choff = np.zeros((NG, W), dtype=np.int64)
    go = 0
    for gi in range(NG):
        for w in range(W):
            choff[gi][w] = go
            go += cap_ch[gi][w]
    NGB = go
    NI = NGB * 128

    # per-(core, chunk, tile) start/end offsets -> shared union block spans
    cumst = np.cumsum(cnt_tg, axis=3) - cnt_tg
    cumen = cumst + cnt_tg
    spans = np.zeros((NG, W, GRP, 2), dtype=np.int64)
    for gi in range(NG):
        for w in range(W):
            for tg in range(GRP):
                has = cnt_tg[:, gi, w, tg] > 0
                if not has.any():
                    continue
                b0 = int(cumst[has, gi, w, tg].min()) // 128
                b1 = -(-int(cumen[has, gi, w, tg].max()) // 128)
                spans[gi, w, tg] = (b0, b1)

    order = np.argsort(key, kind="stable")
    cum = np.zeros(NCORES * NG * W * GRP + 1, dtype=np.int64)
    np.cumsum(cnt_tg.reshape(-1), out=cum[1:])

    idx_all = np.zeros((NCORES, 128, NI // 16), np.int16)
    dstl_all = np.full((NCORES, 128, NGB), -1.0, np.float32)
    i_sorted = i_arr[order]
    dq_sorted = dq_arr[order]
    # per-core dinv by (slot, tile): node at (c, t, slot) has pos (c*T+t)*128+slot
    dv_pos = np.zeros(NT * 128, np.float32)
    dv_pos[pos] = dinv.astype(np.float32)
    dinv_all = np.zeros((NCORES, 128, T), np.float32)
    cnt_all = np.zeros((NCORES, NG * W), np.int32)
    for c in range(NCORES):
        dinv_all[c] = dv_pos[c * T * 128:(c + 1) * T * 128].reshape(T, 128).T
        cnt_all[c] = cnt_ch[c].reshape(-1)
        idx_flat = np.zeros(NI, np.int16)
        dq_flat = np.full(NI, -1.0, np.float32)
        for gi in range(NG):
            for w in range(W):
                k = ((c * NG + gi) * W + w) * GRP
                a, b = cum[k], cum[k + GRP]        # whole chunk, tile-ordered
                n = b - a
                if n == 0:
                    continue
                o = choff[gi][w] * 128
                idx_flat[o:o + n] = i_sorted[a:b]
                dq_flat[o:o + n] = dq_sorted[a:b]
        wrapped = idx_flat.reshape(NI // 16, 16).T          # [16, NI/16]
        idx_all[c] = np.tile(wrapped, (8, 1))
        dstl_all[c] = dq_flat.reshape(NGB, 128).T
    return cap_ch, spans, idx_all, dstl_all, dinv_all, cnt_all, pos


def kernel(x, edge_index, W1, b1, g1, bt1, W2, b2, g2, bt2):
    import ml_dtypes
    from concourse import bass_utils

    x = np.asarray(x, dtype=np.float32)
    cap_ch, spans, idx_all, dstl_all, dinv_all, cnt_all, pos = _preprocess(
        x, np.asarray(edge_index))

    key = cap_ch.tobytes() + spans.tobytes()
    if key not in _CACHE:
        _CACHE[key] = _build_bass(cap_ch, spans)
    nc = _CACHE[key]

    bnp = np.stack([np.asarray(g1, np.float32), np.asarray(bt1, np.float32),
                    np.asarray(g2, np.float32), np.asarray(bt2, np.float32)],
                   axis=1)                                   # [64, 4]
    xp = np.zeros((NP, F), np.float32)
    xp[pos] = x
    in_maps = []
    for c in range(NCORES):
        xs = xp[c * NSP:(c + 1) * NSP]
        in_maps.append({
            "xT": np.ascontiguousarray(xs.T).astype(ml_dtypes.bfloat16),
            "idx": idx_all[c],
            "dstl": np.ascontiguousarray(dstl_all[c]),
            "dinv": np.ascontiguousarray(dinv_all[c]),
            "w1": np.asarray(W1, np.float32).astype(ml_dtypes.bfloat16),
            "w2": np.asarray(W2, np.float32),
            "bnp": bnp,
        })
    kw = {"trace": True} if TRACE else {}
    res = bass_utils.run_bass_kernel_spmd(nc, in_maps,
                                          core_ids=list(range(NCORES)), **kw)
    global LAST
    LAST = res
    big = np.concatenate([res.results[c]["out"] for c in range(NCORES)], axis=0)
    return np.ascontiguousarray(big[pos]).astype(np.float32)



# revision 52
# speedup vs baseline: 1.0076x; 1.0076x over previous
"""2-layer GCN (message passing + BatchNorm + ReLU) on 8 Trainium2 NeuronCores.

Strategy (graph/data parallel, per sharding hint):
- Nodes sharded by dst across 8 cores (12500/core, padded to 12544 = 98 tiles
  of 128). Tiny weights replicated.
- Sym-norm folded into the data path: table rows are pre-scaled by dinv[src]
  (fused into the linear / BN-apply writes), aggregated sums are post-scaled
  by dinv[dst] (fused into the PSUM->SBUF copies via activation scale). The
  per-edge one-hot is then a PURE equality (slot id match), built BATCHED:
  one DVE tensor_tensor(is_equal) per (group, window) chunk produces all of
  the chunk's [128,128] one-hots at once. Self-loops use one shared identity
  matrix (no per-block DVE work at all).
- Aggregation: edges bucketed by (dst-tile, src-window); per 128-edge block,
  gather h[src] rows via dma_gather (256B rows), scatter via accumulating
  matmul in PSUM. int16 gather indices => 4 src windows of 25088.
- BN (training-mode batch stats): per-feature sum/sumsq via one accumulating
  z^T[z|1] matmul (diag = sumsq), cross-core AllReduce of [64,2], affine+ReLU
  applied per tile. Biases b1/b2 cancel under BN and are dropped.
- Layer 2 aggregates first (linearity: segsum(y@W2) = segsum(y)@W2), producing
  accum^T [64,128] per tile, then one matmul with W2 -> node-major output.
- Perf: x/W1 in bf16 (full-rate PE for the input linear); the per-layer table
  AllGather is split into two tile-range halves (windows 0-1 live in half A)
  so windows 0-1 gathers start while half B's collective is in flight; BN
  apply + ReLU + dinv staging run fully batched on the vector engine. The
  dominating cost is gpsimd dma_gather descriptor generation (~8ns/idx on the
  Q7; measured faster than ap_gather/gather_transpose/transpose-mode), which
  bounds this design at ~2.4ms of gpsimd busy time.
"""
import os
import numpy as np
from contextlib import ExitStack

N = 100000
F = 128          # input features
H = 64           # hidden
NCORES = 8
NS = 12500       # nodes per core
T = 98           # dst tiles per core (98*128 = 12544)
NSP = T * 128    # 12544
NP = NCORES * NSP  # padded table rows 100352
WIN = NP // 4    # 25088-row src windows (int16-safe, equal size)
W = 4            # src windows over NP
GRP = 7          # dst tiles per group (PSUM banks: 7 agg + 1 cov)
EPS = 1e-5

_CACHE = {}
TRACE = False     # test-harness hook: profiled run stashes results in LAST
LAST = None


def _build_bass(cap_ch, spans):
    """cap_ch: int array [NG, W] — 128-edge block counts per (tile-group,
    window) chunk, identical across cores. spans[gi, w, tg] = (b0, b1) union
    block range of tile tg's edges within the chunk, across cores."""
    import concourse.bass as bass
    import concourse.tile as tile
    from concourse import bacc, mybir

    f32 = mybir.dt.float32
    f16 = mybir.dt.float16
    bf16 = mybir.dt.bfloat16
    i16 = mybir.dt.int16

    NGB = int(cap_ch.sum())         # gather blocks (no self blocks)
    NI = NGB * 128                  # gathered edge slots
    TA = 49                         # tiles in table half A (half B: T - TA)

    nc = bacc.Bacc("TRN2", target_bir_lowering=False, debug=False,
                   num_devices=NCORES)

    NG = T // GRP
    xT_in = nc.dram_tensor("xT", [F, NSP], bf16, kind="ExternalInput").ap()
    idx_in = nc.dram_tensor("idx", [128, NI // 16], i16, kind="ExternalInput").ap()
    dstl_in = nc.dram_tensor("dstl", [128, NGB], f32, kind="ExternalInput").ap()
    dinv_in = nc.dram_tensor("dinv", [128, T], f32, kind="ExternalInput").ap()
    w1_in = nc.dram_tensor("w1", [F, H], bf16, kind="ExternalInput").ap()
    w2_in = nc.dram_tensor("w2", [H, H], f32, kind="ExternalInput").ap()
    bnp_in = nc.dram_tensor("bnp", [H, 4], f32, kind="ExternalInput").ap()
    out_ext = nc.dram_tensor("out", [NSP, H], f32, kind="ExternalOutput").ap()

    # table split in two tile-range halves so gathers on half A (windows 0-1)
    # can start while half B's AllGather is still in flight
    ag_a = nc.dram_tensor("ag_a", [128, TA * F], f16).ap()
    ag_b = nc.dram_tensor("ag_b", [128, (T - TA) * F], f16).ap()
    table_a = nc.dram_tensor("table_a", [NP // 2, F], f16, addr_space="Shared").ap()
    table_b = nc.dram_tensor("table_b", [NP // 2, F], f16, addr_space="Shared").ap()
    # layer-2 tables are double-buffered: they receive PRE-BN z*dinv rows
    # (dinv stashed in col H) all-gathered DURING layer-1 aggregation; BN+relu
    # is applied per gathered chunk instead (relu(g*sc + of*dinv), dinv >= 0)
    ag_a2 = nc.dram_tensor("ag_a2", [128, TA * F], f16).ap()
    ag_b2 = nc.dram_tensor("ag_b2", [128, (T - TA) * F], f16).ap()
    table_a2 = nc.dram_tensor("table_a2", [NP // 2, F], f16, addr_space="Shared").ap()
    table_b2 = nc.dram_tensor("table_b2", [NP // 2, F], f16, addr_space="Shared").ap()
    ar_in = nc.dram_tensor("ar_in", [H, 2], f32).ap()
    ar_out = nc.dram_tensor("ar_out", [H, 2], f32, addr_space="Shared").ap()

    groups = [list(range(g, min(g + GRP, T))) for g in range(0, T, GRP)]

    # canonical block order: for g: for w: cap_ch[gi][w] blocks (chunk-packed)
    idx_off = {}    # (gi, w) -> (block offset, nblocks)
    goff = 0
    for gi, g in enumerate(groups):
        for w in range(W):
            nb = int(cap_ch[gi][w])
            idx_off[(gi, w)] = (goff, nb)
            goff += nb
    assert goff == NGB

    with tile.TileContext(nc) as tc, ExitStack() as ctx:
        const = ctx.enter_context(tc.tile_pool(name="const", bufs=1))
        big = ctx.enter_context(tc.tile_pool(name="big", bufs=1))
        gp = ctx.enter_context(tc.tile_pool(name="gp", bufs=6))
        ohp = ctx.enter_context(tc.tile_pool(name="ohp", bufs=3))
        ztp = ctx.enter_context(tc.tile_pool(name="ztp", bufs=3))
        ytp = ctx.enter_context(tc.tile_pool(name="ytp", bufs=3))
        smp = ctx.enter_context(tc.tile_pool(name="smp", bufs=2))
        ps_agg = ctx.enter_context(tc.tile_pool(name="ps_agg", bufs=GRP, space="PSUM"))
        ps_cov = ctx.enter_context(tc.tile_pool(name="ps_cov", bufs=1, space="PSUM"))

        # ---- constants / loads
        # qualified-label iota: col c of slice tg holds tg*128 + c, matching
        # the dq = tg*128 + slot one-hot labels
        iota_q = const.tile([128, GRP * 128], f32)
        nc.gpsimd.iota(iota_q[:], pattern=[[1, GRP * 128]], base=0,
                       channel_multiplier=0,
                       allow_small_or_imprecise_dtypes=True)
        # identity [128,128] f16 for self blocks: (c - p == 0)
        iota_d = const.tile([128, 128], f32)
        nc.gpsimd.iota(iota_d[:], pattern=[[1, 128]], base=0, channel_multiplier=-1,
                       allow_small_or_imprecise_dtypes=True)
        ident = const.tile([128, 128], f16)
        nc.vector.tensor_scalar(ident[:], iota_d[:], 0.0, None,
                                op0=mybir.AluOpType.is_equal)
        # diagmask[f, c] = (c - f == 0), [64, 66]; cols 0..63 also serve as I64
        iota_cm = const.tile([H, 66], f32)
        nc.gpsimd.iota(iota_cm[:], pattern=[[1, 66]], base=0, channel_multiplier=-1,
                       allow_small_or_imprecise_dtypes=True)
        diagmask = const.tile([H, 66], f32)
        nc.vector.tensor_scalar(diagmask[:], iota_cm[:], 0.0, None,
                                op0=mybir.AluOpType.is_equal)
        ones1 = const.tile([1, 128], f32)
        nc.vector.memset(ones1[:], 1.0)

        w1_t = const.tile([F, H], bf16)
        nc.sync.dma_start(w1_t[:], w1_in)
        w2_t = const.tile([H, H], f32)
        nc.sync.dma_start(w2_t[:], w2_in)
        bnp_t = const.tile([H, 4], f32)
        nc.sync.dma_start(bnp_t[:], bnp_in)
        dinv_t = const.tile([128, T], f32)
        nc.sync.dma_start(dinv_t[:], dinv_in)
        xT_t = big.tile([F, NSP], bf16, tag="xT")
        nc.sync.dma_start(xT_t[:], xT_in)
        idx_t = big.tile([128, NI // 16], i16, tag="idx")
        nc.sync.dma_start(idx_t[:], idx_in)
        dstl_t = big.tile([128, NGB], f32, tag="dstl")
        nc.sync.dma_start(dstl_t[:], dstl_in)

        tstage = big.tile([128, T, F], f16, tag="tstage")   # table staging
        nc.vector.memset(tstage[:], 0.0)
        nc.vector.tensor_copy(tstage[:, :, H:H + 1], dinv_t[:, :, None])
        z_sb = big.tile([128, T, 66], f32, tag="zsb")       # agg out + ones col
        nc.vector.memset(z_sb[:, :, 64:66], 0.0)
        nc.vector.memset(z_sb[:, :, 64:65], 1.0)

        def ag_half(half, layer=1, part="both"):
            """DMA one tile-range half of tstage out, AllGather it into its
            half table. Half-table row index: r = c*6272 + p*49 + t_local.
            part="dma"/"cc" splits emission: the staging DMA must be emitted
            while tstage still holds the values to ship, but the collective
            trigger (a gpsimd instruction) can be deferred so it doesn't
            head-of-line block the gather stream while its input lands."""
            if layer == 1:
                ag, tbl = (ag_a, table_a) if half == 0 else (ag_b, table_b)
            else:
                ag, tbl = (ag_a2, table_a2) if half == 0 else (ag_b2, table_b2)
            t0, t1 = (0, TA) if half == 0 else (TA, T)
            if part in ("both", "dma"):
                nc.sync.dma_start(ag,
                                  tstage[:, t0:t1, :].rearrange("p t f -> p (t f)"))
            if part in ("both", "cc"):
                nc.gpsimd.collective_compute(
                    "AllGather", mybir.AluOpType.bypass,
                    replica_groups=[list(range(NCORES))],
                    ins=[ag], outs=[tbl],
                )

        def aggregate(layer):
            """Gather + equality-one-hot matmul aggregation.
            layer 1: psum_t [128,64] node-major (lhsT=onehot).
            layer 2: accumT [64,128] (lhsT=messages), then @W2.
            Self-loop: one block per tile, lhsT/rhs = shared identity, rhs/lhsT
            from tstage (rows already dinv[src]-scaled)."""
            cov = ps_cov.tile([H, 66], f32, tag="cov")
            first = [True] * T

            def mm(t, psum_t, layer, oh, rhs_msg, stop):
                if t not in psum_t:
                    psum_t[t] = ps_agg.tile(
                        [128, H] if layer == 1 else [H, 128], f32,
                        tag="agg", name=f"agg_l{layer}_t{t}")
                if layer == 1:
                    nc.tensor.matmul(psum_t[t][:], oh, rhs_msg,
                                     start=first[t], stop=stop)
                else:
                    nc.tensor.matmul(psum_t[t][:], rhs_msg, oh,
                                     start=first[t], stop=stop)
                first[t] = False

            for gi, g in enumerate(groups):
                psum_t = {}
                for w in range(W):
                    o, nb = idx_off[(gi, w)]
                    if nb == 0:
                        continue
                    gt = gp.tile([128, nb, F], f16, tag="g",
                                 name=f"g_l{layer}_{gi}_{w}")
                    if layer == 1:
                        tbl = table_a if w < 2 else table_b
                    else:
                        tbl = table_a2 if w < 2 else table_b2
                    w2 = w % 2
                    assert nb <= 32                 # HW gather cap: 4096 idxs
                    # static full-count gather (runtime reg truncation measured
                    # slower: per-chunk reg_load WAR gating outweighed the
                    # ~4.6% descriptor saving)
                    nc.gpsimd.dma_gather(
                        gt[:, 0:nb, :], tbl[w2 * WIN:(w2 + 1) * WIN, :],
                        idx_t[:, o * 8:(o + nb) * 8],
                        num_idxs=nb * 128, num_idxs_reg=nb * 128,
                        elem_size=F, elem_step=F, single_packet=False)
                    if layer == 2:
                        # per-chunk BN+relu on the raw gathered rows:
                        # msg = relu(g*sc + of*dinv[src]) with dinv from col H
                        msg = ytp.tile([128, nb, H], f16, tag="msg",
                                       name=f"msg_{gi}_{w}")
                        tm2 = ytp.tile([128, nb, H], f16, tag="msg2",
                                       name=f"tm2_{gi}_{w}")
                        nc.vector.tensor_tensor(
                            out=msg[:], in0=gt[:, :, 0:H],
                            in1=bcs[:, None, 0:H].broadcast_to([128, nb, H]),
                            op=mybir.AluOpType.mult)
                        nc.vector.tensor_tensor(
                            out=tm2[:],
                            in0=bcs[:, None, H:128].broadcast_to([128, nb, H]),
                            in1=gt[:, :, H:H + 1].broadcast_to([128, nb, H]),
                            op=mybir.AluOpType.mult)
                        nc.vector.tensor_tensor(
                            out=msg[:], in0=msg[:], in1=tm2[:],
                            op=mybir.AluOpType.add)
                        nc.vector.tensor_scalar(msg[:], msg[:], 0.0, None,
                                                op0=mybir.AluOpType.max)
                    # per-tile one-hots over the tile's union block span;
                    # qualified labels keep other tiles'/pad edges at zero
                    for tg in range(len(g)):
                        b0 = int(spans[gi, w, tg, 0])
                        b1 = int(spans[gi, w, tg, 1])
                        if b1 <= b0:
                            continue
                        sp = b1 - b0
                        oh = ohp.tile([128, sp, 128], f16, tag="oh",
                                      name=f"oh_{gi}_{w}_{tg}")
                        nc.vector.tensor_tensor(
                            out=oh[:],
                            in0=iota_q[:, None, tg * 128:(tg + 1) * 128]
                                .broadcast_to([128, sp, 128]),
                            in1=dstl_t[:, o + b0:o + b1, None]
                                .broadcast_to([128, sp, 128]),
                            op=mybir.AluOpType.is_equal)
                        for b in range(b0, b1):
                            rhs = (gt[:, b, 0:H] if layer == 1
                                   else msg[:, b, :])
                            mm(g[tg], psum_t, layer, oh[:, b - b0, :],
                               rhs, stop=False)
                    if layer == 2 and gi == 0 and w == 0:
                        ag_half(1, layer=2, part="cc")
                for t in g:
                    # self block: identity one-hot, own (scaled) table rows
                    mm(t, psum_t, layer, ident[:], tstage[:, t, 0:H],
                       stop=True)
                for t in g:
                    if layer == 1:
                        nc.scalar.activation(z_sb[:, t, 0:H], psum_t[t][:],
                                             mybir.ActivationFunctionType.Copy,
                                             scale=dinv_t[:, t:t + 1])
                    else:
                        zT = ztp.tile([H, 128], f32, tag="zT",
                                      name=f"zT_{t}")
                        nc.scalar.activation(zT[:], psum_t[t][:],
                                             mybir.ActivationFunctionType.Copy)
                        o2 = ps_agg.tile([128, H], f32, tag="agg",
                                         name=f"o2_{t}")
                        nc.tensor.matmul(o2[:], zT[:], w2_t[:],
                                         start=True, stop=True)
                        nc.scalar.activation(z_sb[:, t, 0:H], o2[:],
                                             mybir.ActivationFunctionType.Copy,
                                             scale=dinv_t[:, t:t + 1])
                    # stats: cov += z^T @ [z | 1]; diag -> sumsq, col 64 -> sum
                    nc.tensor.matmul(cov[:], z_sb[:, t, 0:H], z_sb[:, t, :],
                                     start=(t == 0), stop=(t == T - 1))
                if layer == 1:
                    g0 = g[0]
                    nc.vector.tensor_tensor(
                        out=tstage[:, g0:g0 + len(g), 0:H],
                        in0=z_sb[:, g0:g0 + len(g), 0:H],
                        in1=dinv_t[:, g0:g0 + len(g), None]
                            .broadcast_to([128, len(g), H]),
                        op=mybir.AluOpType.mult)
                    if gi == 6:
                        ag_half(0, layer=2)
                    if gi == len(groups) - 1:
                        arbox.append(stats_ar(cov))
                        ag_half(1, layer=2, part="dma")
            return cov

        def stats_ar(cov):
            """Reduce cov to [sum | sumsq] and AllReduce across cores."""
            cov_sb = smp.tile([H, 66], f32, tag="covsb")
            nc.vector.tensor_copy(cov_sb[:], cov[:])
            stats = smp.tile([H, 2], f32, tag="stats")
            nc.vector.tensor_copy(stats[:, 0:1], cov_sb[:, 64:65])
            masked = smp.tile([H, 66], f32, tag="masked")
            nc.vector.tensor_tensor(out=masked[:], in0=cov_sb[:], in1=diagmask[:],
                                    op=mybir.AluOpType.mult)
            nc.vector.tensor_reduce(out=stats[:, 1:2], in_=masked[:],
                                    op=mybir.AluOpType.add,
                                    axis=mybir.AxisListType.X)
            nc.sync.dma_start(ar_in, stats[:])
            nc.gpsimd.collective_compute(
                "AllReduce", mybir.AluOpType.add,
                replica_groups=[list(range(NCORES))],
                ins=[ar_in], outs=[ar_out])
            ar_sb = smp.tile([H, 2], f32, tag="arsb")
            nc.sync.dma_start(ar_sb[:], ar_out)
            return ar_sb

        def bn_rest(ar_sb, pcol):
            """From all-reduced stats, build bcast psum [128,128]:
            cols 0:64 = scale, 64:128 = offset (per-feature)."""
            m_p = smp.tile([H, 1], f32, tag="m_p")
            nc.vector.tensor_scalar(m_p[:], ar_sb[:, 0:1], 1.0 / N, None,
                                    op0=mybir.AluOpType.mult)
            q_p = smp.tile([H, 1], f32, tag="q_p")
            nc.vector.tensor_scalar(q_p[:], ar_sb[:, 1:2], 1.0 / N, None,
                                    op0=mybir.AluOpType.mult)
            var_p = smp.tile([H, 1], f32, tag="var_p")
            nc.vector.tensor_tensor(out=var_p[:], in0=m_p[:], in1=m_p[:],
                                    op=mybir.AluOpType.mult)
            nc.vector.tensor_tensor(out=var_p[:], in0=q_p[:], in1=var_p[:],
                                    op=mybir.AluOpType.subtract)
            nc.vector.tensor_scalar(var_p[:], var_p[:], EPS, None,
                                    op0=mybir.AluOpType.add)
            std_p = smp.tile([H, 1], f32, tag="std_p")
            nc.scalar.activation(std_p[:], var_p[:],
                                 mybir.ActivationFunctionType.Sqrt)
            rstd_p = smp.tile([H, 1], f32, tag="rstd_p")
            nc.vector.reciprocal(rstd_p[:], std_p[:])
            so_p = smp.tile([H, 2], f32, tag="so_p")
            nc.vector.tensor_tensor(out=so_p[:, 0:1], in0=rstd_p[:],
                                    in1=bnp_t[:, pcol:pcol + 1],
                                    op=mybir.AluOpType.mult)
            nc.vector.tensor_tensor(out=so_p[:, 1:2], in0=m_p[:], in1=so_p[:, 0:1],
                                    op=mybir.AluOpType.mult)
            nc.vector.tensor_tensor(out=so_p[:, 1:2],
                                    in0=bnp_t[:, pcol + 1:pcol + 2],
                                    in1=so_p[:, 1:2],
                                    op=mybir.AluOpType.subtract)
            tr_s = ps_agg.tile([1, H], f32, tag="agg")
            nc.tensor.transpose(tr_s[:], so_p[:, 0:1], diagmask[:, 0:64])
            tr_o = ps_agg.tile([1, H], f32, tag="agg")
            nc.tensor.transpose(tr_o[:], so_p[:, 1:2], diagmask[:, 0:64])
            sc_row = smp.tile([1, H], f32, tag="sc_row")
            nc.vector.tensor_copy(sc_row[:], tr_s[:])
            of_row = smp.tile([1, H], f32, tag="of_row")
            nc.vector.tensor_copy(of_row[:], tr_o[:])
            bc = ps_cov.tile([128, 128], f32, tag="cov")
            nc.tensor.matmul(bc[:, 0:H], ones1[:], sc_row[:],
                             start=True, stop=True)
            nc.tensor.matmul(bc[:, H:128], ones1[:], of_row[:],
                             start=True, stop=True)
            return bc

        # ======== layer 1 ========
        for ha, (h0, h1) in enumerate(((0, TA), (TA, T))):
            for t0 in range(h0, h1, 4):
                k = min(4, h1 - t0)
                mm = ps_agg.tile([128, 4, H], f32, tag="agg", name=f"xmm_{t0}")
                for b in range(k):
                    t = t0 + b
                    nc.tensor.matmul(mm[:, b, :], xT_t[:, t * 128:(t + 1) * 128],
                                     w1_t[:], start=True, stop=True)
                # stage rows pre-scaled by dinv[node]
                nc.vector.tensor_tensor(
                    out=tstage[:, t0:t0 + k, 0:H], in0=mm[:, 0:k, :],
                    in1=dinv_t[:, t0:t0 + k, None].broadcast_to([128, k, H]),
                    op=mybir.AluOpType.mult)
            ag_half(ha)
        arbox = []
        aggregate(1)          # also stages raw z and fires AG2A/AR1/AG2B
        bc1 = bn_rest(arbox.pop(), 0)
        bcs = smp.tile([128, 128], f32, tag="bcs")
        nc.vector.tensor_copy(bcs[:], bc1[:])
        BNC = 7                                   # tiles per batched BN op
        # BN affine + relu + dinv[src] pre-scale, all batched on vector
        # (relu(y)*dinv == relu(y*dinv) since dinv >= 0); per half so the
        # half-A AllGather overlaps half-B's BN apply.
        for ha, (h0, h1) in enumerate(((0, TA), (TA, T))):
            for bi, t0 in enumerate(range(h0, h1, BNC)):
                k = min(BNC, h1 - t0)
                y = ytp.tile([128, BNC, H], f32, tag="y")
                sc_b = bc1[:, None, 0:H].broadcast_to([128, k, H])
                of_b = bc1[:, None, H:128].broadcast_to([128, k, H])
                nc.vector.tensor_tensor(out=y[:, 0:k, :],
                                        in0=z_sb[:, t0:t0 + k, 0:H],
                                        in1=sc_b, op=mybir.AluOpType.mult)
                nc.vector.tensor_tensor(out=y[:, 0:k, :], in0=y[:, 0:k, :],
                                        in1=of_b, op=mybir.AluOpType.add)
                if bi % 2 == 0:
                    # even batches: relu + dinv pre-scale on scalar so the
                    # vector engine streams ahead to the next batch
                    for b in range(k):
                        t = t0 + b
                        nc.scalar.activation(tstage[:, t, 0:H], y[:, b, :],
                                             mybir.ActivationFunctionType.Relu,
                                             scale=dinv_t[:, t:t + 1])
                else:
                    nc.vector.tensor_scalar(y[:, 0:k, :], y[:, 0:k, :], 0.0,
                                            None, op0=mybir.AluOpType.max)
                    nc.vector.tensor_tensor(
                        out=tstage[:, t0:t0 + k, 0:H], in0=y[:, 0:k, :],
                        in1=dinv_t[:, t0:t0 + k, None].broadcast_to([128, k, H]),
                        op=mybir.AluOpType.mult)
        # ======== layer 2 ========
        cov2 = aggregate(2)
        bc2 = bn_rest(stats_ar(cov2), 2)
        out_v = out_ext.rearrange("(t p) f -> t p f", p=128)
        for t0 in range(0, T, BNC):
            k = min(BNC, T - t0)
            y = ytp.tile([128, BNC, H], f32, tag="y")
            sc_b = bc2[:, None, 0:H].broadcast_to([128, k, H])
            of_b = bc2[:, None, H:128].broadcast_to([128, k, H])
            nc.vector.tensor_tensor(out=y[:, 0:k, :], in0=z_sb[:, t0:t0 + k, 0:H],
                                    in1=sc_b, op=mybir.AluOpType.mult)
            nc.vector.tensor_tensor(out=y[:, 0:k, :], in0=y[:, 0:k, :],
                                    in1=of_b, op=mybir.AluOpType.add)
            yo = ytp.tile([128, BNC, H], f32, tag="yo")
            nc.scalar.activation(yo[:, 0:k, :], y[:, 0:k, :],
                                 mybir.ActivationFunctionType.Relu)
            nc.sync.dma_start(out_v[t0:t0 + k].rearrange("t p f -> p t f"),
                              yo[:, 0:k, :])

    nc.compile()
    return nc


def _preprocess(x, edge_index):
    """Shard + bucket edges; build per-core device arrays.

    Nodes are permuted across (core, tile, slot) by snake-dealing in
    decreasing in-degree order, equalizing per-tile edge loads so the
    shared-across-cores bucket caps carry minimal padding. Self-loops are
    NOT bucketed: each tile gets one identity-matmul self block. Sym-norm
    is folded into per-node dinv scales (table rows pre-scaled by dinv[src],
    aggregated output post-scaled by dinv[dst])."""
    src = np.asarray(edge_index[0], dtype=np.int64)
    dst = np.asarray(edge_index[1], dtype=np.int64)
    deg = (np.bincount(dst, minlength=N) + 1).astype(np.float64)  # + self loop
    dinv = 1.0 / np.sqrt(deg)

    # ---- balance: node -> padded global position (tile*128 + slot)
    NT = NCORES * T                               # 784 tiles
    order = np.argsort(-deg, kind="stable")
    pos = np.empty(N, dtype=np.int64)
    for r in range((N + NT - 1) // NT):
        chunk = order[r * NT:(r + 1) * NT]
        tiles = np.arange(len(chunk))
        if r % 2 == 1:
            tiles = NT - 1 - tiles
        pos[chunk] = tiles * 128 + r
    # split-table row of a node (half A: t<49, half B: t>=49):
    # r_half = c*6272 + slot*49 + t_local; windows 0-1 in A, 2-3 in B
    TA = 49
    tile_of = pos >> 7
    slot_of = pos & 127
    core_of = tile_of // T
    t_of = tile_of - core_of * T
    in_b = t_of >= TA
    t_loc = np.where(in_b, t_of - TA, t_of)
    half_row = core_of * (TA * 128) + slot_of * TA + t_loc
    sp = half_row[src]                             # row within half table
    sw = (in_b[src].astype(np.int64) * 2) + sp // WIN  # window 0-3
    dp = pos[dst]
    core = dp // NSP
    local = dp - core * NSP
    t_arr = local >> 7
    dl_arr = (local & 127).astype(np.float32)
    w_arr = sw
    i_arr = (sp % WIN).astype(np.int16)

    # chunk-packed buckets: chunk = (group gi, window w); edges packed
    # tile-ordered with pads only at the chunk tail. Block quantization then
    # amortizes over ~2300-edge chunks (~9% pad) instead of ~320-edge
    # (tile, w) buckets (~20% pad). Boundary blocks span 2 tiles; the matmul
    # schedule uses the UNION over cores of each tile's block range, with
    # tile-qualified one-hot labels dq = tg*128 + slot.
    NG = T // GRP                                  # 14 groups
    g_arr = t_arr // GRP
    tg_arr = t_arr % GRP
    dq_arr = (tg_arr * 128 + (local & 127)).astype(np.float32)
    key = (((core * NG + g_arr) * W + w_arr) * GRP + tg_arr).astype(np.int64)
    cnt_tg = np.bincount(key, minlength=NCORES * NG * W * GRP) \
        .reshape(NCORES, NG, W, GRP)
    cnt_ch = cnt_tg.sum(axis=3)                    # [c, NG, W]
    cap_ch = ((cnt_ch.max(axis=0) + 127) // 128).astype(np.int64)  # [NG, W]
    choff = np.zeros((NG, W), dtype=np.int64)
    go = 0
    for gi in range(NG):
        for w in range(W):
            choff[gi][w] = go
            go += cap_ch[gi][w]
    NGB = go
    NI = NGB * 128

    # per-(core, chunk, tile) start/end offsets -> shared union block spans
    cumst = np.cumsum(cnt_tg, axis=3) - cnt_tg
    cumen = cumst + cnt_tg
    spans = np.zeros((NG, W, GRP, 2), dtype=np.int64)
    for gi in range(NG):
        for w in range(W):
            for tg in range(GRP):
                has = cnt_tg[:, gi, w, tg] > 0
                if not has.any():
                    continue
                b0 = int(cumst[has, gi, w, tg].min()) // 128
                b1 = -(-int(cumen[has, gi, w, tg].max()) // 128)
                spans[gi, w, tg] = (b0, b1)

    order = np.argsort(key, kind="stable")
    cum = np.zeros(NCORES * NG * W * GRP + 1, dtype=np.int64)
    np.cumsum(cnt_tg.reshape(-1), out=cum[1:])

    idx_all = np.zeros((NCORES, 128, NI // 16), np.int16)
    dstl_all = np.full((NCORES, 128, NGB), -1.0, np.float32)
    i_sorted = i_arr[order]
    dq_sorted = dq_arr[order]
    # per-core dinv by (slot, tile): node at (c, t, slot) has pos (c*T+t)*128+slot
    dv_pos = np.zeros(NT * 128, np.float32)
    dv_pos[pos] = dinv.astype(np.float32)
    dinv_all = np.zeros((NCORES, 128, T), np.float32)
    cnt_all = np.zeros((NCORES, NG * W), np.int32)
    for c in range(NCORES):
        dinv_all[c] = dv_pos[c * T * 128:(c + 1) * T * 128].reshape(T, 128).T
        cnt_all[c] = cnt_ch[c].reshape(-1)
        idx_flat = np.zeros(NI, np.int16)
        dq_flat = np.full(NI, -1.0, np.float32)
        for gi in range(NG):
            for w in range(W):
                k = ((c * NG + gi) * W + w) * GRP
                a, b = cum[k], cum[k + GRP]        # whole chunk, tile-ordered
                n = b - a
                if n == 0:
                    continue
                o = choff[gi][w] * 128
                idx_flat[o:o + n] = i_sorted[a:b]
                dq_flat[o:o + n] = dq_sorted[a:b]
        wrapped = idx_flat.reshape(NI // 16, 16).T          # [16, NI/16]
        idx_all[c] = np.tile(wrapped, (8, 1))
        dstl_all[c] = dq_flat.reshape(NGB, 128).T
    return cap_ch, spans, idx_all, dstl_all, dinv_all, cnt_all, pos


def kernel(x, edge_index, W1, b1, g1, bt1, W2, b2, g2, bt2):
    import ml_dtypes
    from concourse import bass_utils

    x = np.asarray(x, dtype=np.float32)
    cap_ch, spans, idx_all, dstl_all, dinv_all, cnt_all, pos = _preprocess(
        x, np.asarray(edge_index))

    key = cap_ch.tobytes() + spans.tobytes()
    if key not in _CACHE:
        _CACHE[key] = _build_bass(cap_ch, spans)
    nc = _CACHE[key]

    bnp = np.stack([np.asarray(g1, np.float32), np.asarray(bt1, np.float32),
                    np.asarray(g2, np.float32), np.asarray(bt2, np.float32)],
                   axis=1)                                   # [64, 4]
    xp = np.zeros((NP, F), np.float32)
    xp[pos] = x
    in_maps = []
    for c in range(NCORES):
        xs = xp[c * NSP:(c + 1) * NSP]
        in_maps.append({
            "xT": np.ascontiguousarray(xs.T).astype(ml_dtypes.bfloat16),
            "idx": idx_all[c],
            "dstl": np.ascontiguousarray(dstl_all[c]),
            "dinv": np.ascontiguousarray(dinv_all[c]),
            "w1": np.asarray(W1, np.float32).astype(ml_dtypes.bfloat16),
            "w2": np.asarray(W2, np.float32),
            "bnp": bnp,
        })
    kw = {"trace": True} if TRACE else {}
    res = bass_utils.run_bass_kernel_spmd(nc, in_maps,
                                          core_ids=list(range(NCORES)), **kw)
    global LAST
    LAST = res
    big = np.concatenate([res.results[c]["out"] for c in range(NCORES)], axis=0)
    return np.ascontiguousarray(big[pos]).astype(np.float32)

